# revision 1
# baseline (speedup 1.0000x reference)
"""CAB (channel-attention block) Trainium2 kernel.

Sharding: 8 cores = 4 batches x 2 H-halves. Each core computes its
[64, 128, 256] slice of the output. The q.kT contraction and the L2-norm
sums are AllReduced across the 2 cores sharing a batch (sequence-parallel).

Math folds used (all exact):
  - L2 normalize folds into S: attn_logits = S * temp / (||q|| ||k||^T),
    with S = q @ k^T computed on raw (unnormalized) q, k.
  - proj o (attn @ v) o dwconv_v o pwconv_v folds into a single 3x3 dense
    conv on input2 with data-dependent matrices
    G[dp] = (P @ A_blockdiag) @ (diag(wdv[:,dp]) @ Wv).
"""
import sys

sys.path.insert(0, "/opt/trn_rl_repo")

import numpy as np

import concourse.bacc as bacc
import concourse.bass as bass
import concourse.tile as tile
from concourse import mybir
from concourse.bass_utils import run_bass_kernel_spmd

F32 = mybir.dt.float32

B, C, H, W = 4, 64, 256, 256
HEADS = 8
HD = C // HEADS
EPS = 1e-12

HH = H // 2            # rows per core
R = W + 2              # padded row length
NR = HH + 4            # buffer rows: zero, halo, 128 data, halo, zero
NBUF = NR * R          # per-core padded input length (flattened)
P0 = 2 * R             # first output position (data row 0, col -1(pad))
NOUT = HH * R          # output span incl. per-row col pads

TAPS = [(dy, dx) for dy in (-1, 0, 1) for dx in (-1, 0, 1)]
# rhs offset of tap within a window that starts 259 cols before the chunk
TAP_OFF = [259 + dy * R + dx for dy, dx in TAPS]

SPAN1 = 2048           # pass-1 window span (multiple of 512)
ROWS2 = 8              # pass-2 window rows

_CACHE = {}


def _pad_positions(start, length):
    """Contiguous runs of pad columns (global col % R in {0, R-1}) within
    [start, start+length), as (offset_rel, run_len) with runs <= 2."""
    runs = []
    g = start
    end = start + length
    # pads occur at positions p with p % R == 0 or R-1; adjacent pairs.
    p = (start // R) * R - 1
    while p < end:
        for q in (p, p + 1):  # (row r col 257, row r+1 col 0) adjacent pair
            lo = max(q, start)
            hi = min(q + 1, end)
            if lo < hi:
                if runs and runs[-1][0] + runs[-1][1] == lo - start:
                    runs[-1] = (runs[-1][0], runs[-1][1] + (hi - lo))
                else:
                    runs.append((lo - start, hi - lo))
        p += R
    return runs


def build_module(mode="full"):
    nc = bacc.Bacc("TRN2", target_bir_lowering=False, debug=False, num_devices=8)

    x1 = nc.declare_dram_parameter("x1", [C, NBUF], F32, isOutput=False)
    x2 = nc.declare_dram_parameter("x2", [C, NBUF], F32, isOutput=False)
    lqkT = nc.declare_dram_parameter("lqkT", [128, 9 * C], F32, isOutput=False)
    wv9 = nc.declare_dram_parameter("wv9", [C, 9 * C], F32, isOutput=False)
    pT = nc.declare_dram_parameter("pT", [C, C], F32, isOutput=False)
    temp64 = nc.declare_dram_parameter("temp64", [C, 1], F32, isOutput=False)
    ident = nc.declare_dram_parameter("ident", [128, 128], F32, isOutput=False)
    mask64 = nc.declare_dram_parameter("mask64", [C, C], F32, isOutput=False)
    y = nc.declare_dram_parameter("y", [C, HH, W], F32, isOutput=True)

    with tile.TileContext(nc) as tc:
        _body(tc, nc, x1, x2, lqkT, wv9, pT, temp64, ident, mask64, y, mode)
    nc.compile()
    return nc


def _body(tc, nc, x1, x2, lqkT, wv9, pT, temp64, ident, mask64, y, mode="full"):
    mm = nc.tensor.matmul
    f = F32

    wpool = tc.alloc_tile_pool(name="weights", bufs=1)
    dram = tc.alloc_tile_pool(name="dram", bufs=1, space="DRAM")
    accp = tc.alloc_tile_pool(name="ps_acc", bufs=1, space=bass.MemorySpace.PSUM)
    persist = tc.alloc_tile_pool(name="persist", bufs=1)

    w_lqkT = wpool.tile([128, 9 * C], f)
    nc.gpsimd.dma_start(w_lqkT[:], lqkT[:])
    w_wv9 = wpool.tile([C, 9 * C], f)
    nc.gpsimd.dma_start(w_wv9[:], wv9[:])
    w_pT = wpool.tile([C, C], f)
    nc.gpsimd.dma_start(w_pT[:], pT[:])
    w_temp = wpool.tile([C, 1], f)
    nc.gpsimd.dma_start(w_temp[:], temp64[:])
    w_id = wpool.tile([128, 128], f)
    nc.gpsimd.dma_start(w_id[:], ident[:])
    w_mask = wpool.tile([C, C], f)
    nc.gpsimd.dma_start(w_mask[:], mask64[:])

    acc_ps = accp.tile([C, C], f)          # S accumulator (q.kT)
    qk2 = persist.tile([128, 1], f)        # running sum q^2 (top) / k^2 (bottom)
    nc.vector.memset(qk2[:], 0.0)

    # ---------------- pass 1: q,k conv -> transpose -> S, norms ----------
    n_sub_total = NOUT // 128
    sub_idx = 0
    with (
        tc.tile_pool(name="xw1", bufs=2) as xw1p,
        tc.tile_pool(name="qkwin", bufs=2) as qkwp,
        tc.tile_pool(name="trsb", bufs=3) as trsbp,
        tc.tile_pool(name="scratch", bufs=1) as scrp,
        tc.tile_pool(name="ps_conv", bufs=2, space=bass.MemorySpace.PSUM) as pcv,
        tc.tile_pool(name="ps_tr", bufs=2, space=bass.MemorySpace.PSUM) as ptr,
    ):
        scratch = scrp.tile([128, SPAN1], f)
        acc_tmp = scrp.tile([128, 1], f, tag="acctmp")
        for wstart in range(0, NOUT, SPAN1):
            width = min(SPAN1, NOUT - wstart)
            p_start = P0 + wstart
            ws = p_start - 259
            wwidth = width + 518
            xw = xw1p.tile([128, SPAN1 + 518], f)
            nc.gpsimd.dma_start(xw[0:C, 0:wwidth], x1[:, ws:ws + wwidth])
            nc.gpsimd.dma_start(xw[C:128, 0:wwidth], x2[:, ws:ws + wwidth])

            qkwin = qkwp.tile([128, SPAN1], f)
            for lc in range(0, width, 512):
                L = min(512, width - lc)
                # q and k accumulate in separate banks (own psum groups);
                # k writes partitions 64:128 so SBUF copies stay aligned.
                qps = pcv.tile([128, 512], f, tag="qps")
                kps = pcv.tile([128, 512], f, tag="kps")
                for t in range(9):
                    o = lc + TAP_OFF[t]
                    mm(qps[0:C, 0:L], w_lqkT[0:C, t * C:(t + 1) * C],
                       xw[0:C, o:o + L], start=(t == 0), stop=(t == 8),
                       tile_position=(0, 0))
                    mm(kps[C:128, 0:L], w_lqkT[C:128, t * C:(t + 1) * C],
                       xw[C:128, o:o + L], start=(t == 0), stop=(t == 8),
                       tile_position=(64, 64))
                nc.scalar.copy(qkwin[0:C, lc:lc + L], qps[0:C, 0:L])
                nc.scalar.copy(qkwin[C:128, lc:lc + L], kps[C:128, 0:L])

            # zero the per-row pad columns so they don't pollute S / norms
            if mode != "conv":
                for off, ln in _pad_positions(p_start, width):
                    nc.gpsimd.memset(qkwin[:, off:off + ln], 0.0)

            # norms: accumulate sum of squares over this window
            if mode not in ("conv", "convtr"):
                nc.scalar.activation(
                    scratch[:, 0:width], qkwin[:, 0:width],
                    mybir.ActivationFunctionType.Square, accum_out=acc_tmp[:])
                nc.vector.tensor_add(qk2[:], qk2[:], acc_tmp[:])

            # S += Tq.T @ Tk per 128-col sub-chunk
            if mode not in ("conv", "convttr"):
                for j in range(0, width, 128):
                    trps = ptr.tile([128, 128], f, tag="trps")
                    nc.tensor.transpose(trps[:], qkwin[:, j:j + 128], w_id[:])
                    trsb = trsbp.tile([128, 128], f)
                    nc.scalar.copy(trsb[:], trps[:])
                    mm(acc_ps[:], trsb[:, 0:C], trsb[:, C:128],
                       start=(sub_idx == 0), stop=(sub_idx == n_sub_total - 1))
                    sub_idx += 1

        if mode == "conv":
            # anchor so DCE can't drop the conv chain
            nc.sync.dma_start(y[:, 1, 0:C], qkwin[0:C, 0:C])

    # ---------------- collective: S and norms over the batch pair --------
    cc_sb = persist.tile([128, C + 1], f)
    nc.vector.memset(cc_sb[:], 0.0)
    if mode in ("conv", "convttr"):
        nc.scalar.copy(cc_sb[0:C, 0:C], w_pT[:])
    else:
        nc.scalar.copy(cc_sb[0:C, 0:C], acc_ps[:])
    nc.vector.tensor_copy(cc_sb[:, C:C + 1], qk2[:])
    if mode in ("conv", "convttr", "convtr"):
        nc.sync.dma_start(y[:, 0, 0:C + 1], cc_sb[0:C, :])
        for p in (persist, dram, wpool):
            p.release()
        accp.release()
        return
    cc_in = dram.tile([128, C + 1], f)
    cc_out = dram.tile([128, C + 1], f, tag="cc_out")
    nc.sync.dma_start(cc_in[:], cc_sb[:])
    if mode == "p1":
        nc.sync.dma_start(y[:, 0, 0:C + 1], cc_sb[0:C, :])
        for p in (persist, dram, wpool):
            p.release()
        accp.release()
        return
    if mode == "nocc":
        nc.gpsimd.dma_start(cc_out[:], cc_in[:])
    else:
        nc.gpsimd.collective_compute(
            "AllReduce", mybir.AluOpType.add,
            replica_groups=[[0, 1], [2, 3], [4, 5], [6, 7]],
            ins=[cc_in.opt()], outs=[cc_out.opt()],
        )
    sqk = persist.tile([128, C + 1], f, tag="sqk")
    nc.sync.dma_start(sqk[:], cc_out[:])
    if mode == "p1cc":
        nc.sync.dma_start(y[:, 0, 0:C + 1], sqk[0:C, :])
        for p in (persist, dram, wpool):
            p.release()
        accp.release()
        return

    # ---------------- tiny mid-section: softmax, M^T, G^T ----------------
    with (
        tc.tile_pool(name="mid", bufs=1) as midp,
        tc.tile_pool(name="ps_mid", bufs=1, space=bass.MemorySpace.PSUM) as pmid,
    ):
        nrm = midp.tile([128, 1], f, tag="nrm")       # sqrt of sums
        nc.scalar.sqrt(nrm[:], sqk[:, C:C + 1])
        nc.vector.tensor_scalar_max(nrm[:], nrm[:], EPS)
        rn = midp.tile([128, 1], f, tag="rn")         # 1/||.||
        nc.vector.reciprocal(rn[:], nrm[:])
        rs = midp.tile([C, 1], f, tag="rs")           # temp/||q|| per row c
        nc.vector.tensor_mul(rs[:], rn[0:C, :], w_temp[:])

        # broadcast 1/||k|| along free dim: transpose then rank-1 outer
        nkT_ps = pmid.tile([1, C], f, tag="nkT")
        nc.tensor.transpose(nkT_ps[:], rn[C:128, :], w_id[C:128, C:128])
        nkT = midp.tile([1, C], f, tag="nkT_sb")
        nc.scalar.copy(nkT[:], nkT_ps[:])
        ones1 = midp.tile([1, C], f, tag="ones1")
        nc.vector.memset(ones1[:], 1.0)
        nkb_ps = pmid.tile([C, C], f, tag="nkb")
        mm(nkb_ps[:], ones1[:], nkT[:])
        # logits = S * rs(row) * (1/||k||)(col)
        sp = midp.tile([C, C], f, tag="sp")
        nc.vector.tensor_scalar(sp[:], sqk[0:C, 0:C], rs[:], None,
                                op0=mybir.AluOpType.mult)
        nc.vector.tensor_mul(sp[:], sp[:], nkb_ps[:])

        # blockwise softmax via additive off-block mask (-1e30):
        # off-block entries exp to exactly 0, so the result IS Ablk.
        nc.vector.tensor_add(sp[:], sp[:], w_mask[:])
        negm = midp.tile([C, 1], f, tag="negm")
        nc.vector.tensor_reduce(negm[:], sp[:], axis=mybir.AxisListType.X,
                                op=mybir.AluOpType.max, negate=True)
        den = midp.tile([C, 1], f, tag="den")
        ex = midp.tile([C, C], f, tag="ex")
        nc.scalar.activation(ex[:], sp[:], mybir.ActivationFunctionType.Exp,
                             bias=negm[:], scale=1.0, accum_out=den[:])
        rden = midp.tile([C, 1], f, tag="rden")
        nc.vector.reciprocal(rden[:], den[:])
        ablk = midp.tile([C, C], f, tag="ablk")
        nc.vector.tensor_scalar(ablk[:], ex[:], rden[:], None,
                                op0=mybir.AluOpType.mult)

        # M^T = Ablk^T @ P^T
        mt_ps = pmid.tile([C, C], f, tag="mt")
        mm(mt_ps[:], ablk[:], w_pT[:])
        mt = midp.tile([C, C], f, tag="mt_sb")
        nc.scalar.copy(mt[:], mt_ps[:])

        # G^T[dp] = wv9[dp] @ M^T, duplicated to partitions 64:128
        gt_ps = pmid.tile([128, 9 * C], f, tag="gt")
        for dp in range(9):
            s = dp * C
            mm(gt_ps[0:C, s:s + C], w_wv9[:, s:s + C], mt[:],
               tile_position=(0, 0))
            mm(gt_ps[C:128, s:s + C], w_wv9[:, s:s + C], mt[:],
               tile_position=(0, 64))
        gt = persist.tile([128, 9 * C], f, tag="gt_sb")
        nc.scalar.copy(gt[:], gt_ps[:])

    accp.release()

    # ---------------- pass 2: out = G-conv(x2), write y ------------------
    with (
        tc.tile_pool(name="xw2", bufs=2) as xw2p,
        tc.tile_pool(name="osb", bufs=3) as osbp,
        tc.tile_pool(name="ps_p2", bufs=2, space=bass.MemorySpace.PSUM) as pp2,
    ):
        for rr in range(0, HH, ROWS2):
            nrows = min(ROWS2, HH - rr)
            p_start = P0 + rr * R
            ws = p_start - 259
            wwidth = nrows * R + 518
            xw = xw2p.tile([128, ROWS2 * R + 518], f)
            nc.gpsimd.dma_start(xw[0:C, 0:wwidth], x2[:, ws:ws + wwidth])
            nc.gpsimd.dma_start(xw[C:128, 0:wwidth], x2[:, ws:ws + wwidth])
            for r in range(nrows):
                base = r * R
                o2a = pp2.tile([C, R], f, tag="o2a")
                o2b = pp2.tile([C, R], f, tag="o2b")
                for t in range(9):
                    o = base + TAP_OFF[t]
                    if t % 2 == 0:
                        mm(o2a[:], gt[0:C, t * C:(t + 1) * C],
                           xw[0:C, o:o + R], start=(t == 0), stop=(t == 8),
                           tile_position=(0, 0))
                    else:
                        mm(o2b[:], gt[C:128, t * C:(t + 1) * C],
                           xw[C:128, o:o + R], start=(t == 1), stop=(t == 7),
                           tile_position=(64, 0))
                obs = osbp.tile([C, R], f, tag="obs")
                nc.scalar.copy(obs[:], o2b[:])
                osb = osbp.tile([C, R], f)
                nc.vector.tensor_add(osb[:], o2a[:], obs[:])
                nc.sync.dma_start(y[:, rr + r, :], osb[:, 1:W + 1])

    for p in (persist, dram, wpool):
        p.release()


# ======================= host side =========================================

def _prep_consts(q_w, q_dw_w, kv_w, kv_dw_w, proj_w, temperature):
    q_w = np.asarray(q_w, np.float32)[:, :, 0, 0]          # [o, i]
    kv_w = np.asarray(kv_w, np.float32)[:, :, 0, 0]        # [2C, i]
    q_dw = np.asarray(q_dw_w, np.float32)[:, 0]            # [C, 3, 3]
    kv_dw = np.asarray(kv_dw_w, np.float32)[:, 0]          # [2C, 3, 3]
    proj = np.asarray(proj_w, np.float32)[:, :, 0, 0]      # [o, c]
    temp = np.asarray(temperature, np.float32).reshape(HEADS)

    lqkT = np.zeros((128, 9 * C), np.float32)
    wv9 = np.zeros((C, 9 * C), np.float32)
    for t, (dy, dx) in enumerate(TAPS):
        w9q = q_dw[:, dy + 1, dx + 1][:, None] * q_w       # [o, i]
        w9k = kv_dw[0:C, dy + 1, dx + 1][:, None] * kv_w[0:C]
        lqkT[0:C, t * C:(t + 1) * C] = w9q.T
        lqkT[C:128, t * C:(t + 1) * C] = w9k.T
        # wv9[dp][d, i] = wdv[d, dp] * Wv[d, i]
        wv9[:, t * C:(t + 1) * C] = (
            kv_dw[C:2 * C, dy + 1, dx + 1][:, None] * kv_w[C:2 * C]
        )
    pTm = proj.T.copy()                                    # [c, o]
    temp64 = np.repeat(temp, HD).reshape(C, 1).astype(np.float32)
    ident = np.eye(128, dtype=np.float32)
    mask = np.full((C, C), -1e30, np.float32)
    for h in range(HEADS):
        mask[h * HD:(h + 1) * HD, h * HD:(h + 1) * HD] = 0.0
    return lqkT, wv9, pTm, temp64, ident, mask


def _prep_slice(img):
    """[C, H, W] -> padded flat [C, NBUF] per half; returns (top, bottom)."""
    out = []
    for h in range(2):
        buf = np.zeros((C, NR, R), np.float32)
        r0 = h * HH
        lo, hi = r0 - 1, r0 + HH + 1
        vlo, vhi = max(lo, 0), min(hi, H)
        buf[:, 1 + (vlo - lo):1 + (vlo - lo) + (vhi - vlo), 1:W + 1] = \
            img[:, vlo:vhi, :]
        out.append(np.ascontiguousarray(buf.reshape(C, NBUF)))
    return out


def kernel(input1, input2, q_w, q_dw_w, kv_w, kv_dw_w, proj_w, temperature):
    if "nc" not in _CACHE:
        _CACHE["nc"] = build_module()
    nc = _CACHE["nc"]

    lqkT, wv9, pTm, temp64, ident, mask = _prep_consts(
        q_w, q_dw_w, kv_w, kv_dw_w, proj_w, temperature)

    input1 = np.asarray(input1, np.float32)
    input2 = np.asarray(input2, np.float32)
    in_maps = []
    for core in range(8):
        b, h = core // 2, core % 2
        x1t = _prep_slice(input1[b])[h]
        x2t = _prep_slice(input2[b])[h]
        in_maps.append({
            "x1": x1t, "x2": x2t, "lqkT": lqkT, "wv9": wv9, "pT": pTm,
            "temp64": temp64, "ident": ident, "mask64": mask,
        })

    results = _get_runner(nc)(in_maps)
    out = np.empty((B, C, H, W), np.float32)
    for core in range(8):
        b, h = core // 2, core % 2
        out[b, :, h * HH:(h + 1) * HH, :] = results[core]["y"]
    return out


def _get_runner(nc, n_cores=8):
    """Like bass2jax.run_bass_via_pjrt, but the jitted shard_map executable is
    built once and reused across calls (avoids per-call retracing)."""
    if "runner" in _CACHE:
        return _CACHE["runner"]
    import jax
    from jax.sharding import Mesh, PartitionSpec
    from jax.experimental.shard_map import shard_map
    from concourse import bass2jax as b2j
    from concourse import mybir as _mb

    b2j.install_neuronx_cc_hook()
    partition_name = nc.partition_id_tensor.name if nc.partition_id_tensor else None
    in_names, out_names, out_avals, zero_shapes = [], [], [], []
    for alloc in nc.m.functions[0].allocations:
        if not isinstance(alloc, _mb.MemoryLocationSet):
            continue
        name = alloc.memorylocations[0].name
        if alloc.kind == "ExternalInput":
            if name != partition_name:
                in_names.append(name)
        elif alloc.kind == "ExternalOutput":
            out_names.append(name)
            shape = tuple(alloc.tensor_shape)
            dtype = _mb.dt.np(alloc.dtype)
            out_avals.append(jax.core.ShapedArray(shape, dtype))
            zero_shapes.append((shape, dtype))
    n_params = len(in_names)
    n_outs = len(out_avals)
    all_in_names = list(in_names) + list(out_names)
    if partition_name is not None:
        all_in_names.append(partition_name)
    donate = tuple(range(n_params, n_params + n_outs))

    def _pjrt_body(*args):
        operands = list(args)
        if partition_name is not None:
            operands.append(b2j.partition_id_tensor())
        return tuple(b2j._bass_exec_p.bind(
            *operands, out_avals=tuple(out_avals), in_names=tuple(all_in_names),
            out_names=tuple(out_names), lowering_input_output_aliases=(),
            sim_require_finite=True, sim_require_nnan=True, nc=nc))

    devices = jax.devices()[:n_cores]
    mesh = Mesh(np.asarray(devices), ("core",))
    sharded = jax.jit(
        shard_map(_pjrt_body, mesh=mesh,
                  in_specs=(PartitionSpec("core"),) * (n_params + n_outs),
                  out_specs=(PartitionSpec("core"),) * n_outs, check_rep=False),
        donate_argnums=donate, keep_unused=True)

    def run(in_maps):
        concat_in = [
            np.concatenate([np.asarray(in_maps[c][nm]) for c in range(n_cores)], 0)
            for nm in in_names
        ]
        concat_zeros = [np.zeros((n_cores * s[0], *s[1:]), d)
                        for s, d in zero_shapes]
        out_arrs = sharded(*concat_in, *concat_zeros)
        return [
            {nm: np.asarray(out_arrs[i]).reshape(n_cores, *out_avals[i].shape)[c]
             for i, nm in enumerate(out_names)}
            for c in range(n_cores)
        ]

    _CACHE["runner"] = run
    return run



# revision 3
# speedup vs baseline: 1.1833x; 1.1833x over previous
"""CAB (channel-attention block) Trainium2 kernel.

Sharding: 8 cores = 4 batches x 2 H-halves (mesh ('b','h') = (4,2)).
Inputs are passed RAW ([4,C,256,256] f32) and sharded by jax directly onto
the cores (no host-side padding/copies). Each core DMAs its [C,128,256]
slice row-window by row-window into padded SBUF tiles (guard col + 258-col
padded rows, pads pre-zeroed once on persistent tiles). One-row halos come
from a tiny host-built [8,C,4,W] tensor.

Math folds used (all exact):
  - L2 normalize folds into S: attn_logits = S * temp / (||q|| ||k||^T),
    with S = q @ k^T computed on raw (unnormalized) q, k.
  - proj o (attn @ v) o dwconv_v o pwconv_v folds into a single 3x3 dense
    conv on input2 with data-dependent matrices
    G[dp] = (P @ A_blockdiag) @ (diag(wdv[:,dp]) @ Wv).
The q.kT contraction and the L2-norm sums are AllReduced across the 2
cores sharing a batch (sequence-parallel).
"""
import sys

sys.path.insert(0, "/opt/trn_rl_repo")

import numpy as np

import concourse.bacc as bacc
import concourse.bass as bass
import concourse.tile as tile
from concourse import mybir

F32 = mybir.dt.float32

B, C, H, W = 4, 64, 256, 256
HEADS = 8
HD = C // HEADS
EPS = 1e-12

HH = H // 2            # rows per core
R = W + 2              # padded row length
WIN = 8                # output rows per window
NW = HH // WIN         # windows per pass
SPAN = WIN * R         # 2064 output positions per window (incl row pads)
QKW = 2176             # qkwin width: SPAN rounded up to 17*128 (tail zeroed)
NSUB = QKW // 128      # S sub-chunks per window
XT = 1 + 10 * R + 1    # xw tile: guard col + 10 padded rows + spare

TAPS = [(dy, dx) for dy in (-1, 0, 1) for dx in (-1, 0, 1)]
# rhs offset of tap for pass-1 chunks (output j=0 is row lr=1 col-pad at
# tile position 259; tap reads 259 + dy*258 + dx >= 0)
TAP_OFF = [259 + dy * R + dx for dy, dx in TAPS]

_CACHE = {}


def build_module():
    nc = bacc.Bacc("TRN2", target_bir_lowering=False, debug=False, num_devices=8)

    x1 = nc.declare_dram_parameter("x1", [1, C, HH, W], F32, isOutput=False)
    x2 = nc.declare_dram_parameter("x2", [1, C, HH, W], F32, isOutput=False)
    hal = nc.declare_dram_parameter("hal", [1, C, 4, W], F32, isOutput=False)
    lqkT = nc.declare_dram_parameter("lqkT", [128, 9 * C], F32, isOutput=False)
    wv9 = nc.declare_dram_parameter("wv9", [C, 9 * C], F32, isOutput=False)
    pT = nc.declare_dram_parameter("pT", [C, C], F32, isOutput=False)
    temp64 = nc.declare_dram_parameter("temp64", [C, 1], F32, isOutput=False)
    ident = nc.declare_dram_parameter("ident", [128, 128], F32, isOutput=False)
    mask64 = nc.declare_dram_parameter("mask64", [C, C], F32, isOutput=False)
    y = nc.declare_dram_parameter("y", [1, C, HH, W], F32, isOutput=True)

    with tile.TileContext(nc) as tc:
        _body(tc, nc, x1, x2, hal, lqkT, wv9, pT, temp64, ident, mask64, y)
    nc.compile()
    return nc


def _load_window(nc, xwin, w, src_a, src_b, hal, hrow_a, hrow_b):
    """DMA raw rows [8w-1, 8w+9) of src_a -> partitions 0:64 and src_b ->
    64:128 of the padded window tile, halos from hal rows hrow_a/hrow_b."""
    rr = WIN * w
    g0, g1 = max(0, rr - 1), min(HH, rr + 9)
    l0 = g0 - (rr - 1)
    d3 = xwin[:, 1:1 + 10 * R].rearrange("p (r c) -> p r c", c=R)
    nc.gpsimd.dma_start(d3[0:C, l0:l0 + g1 - g0, 1:W + 1], src_a[0, :, g0:g1, :])
    nc.gpsimd.dma_start(d3[C:128, l0:l0 + g1 - g0, 1:W + 1], src_b[0, :, g0:g1, :])
    if rr == 0:
        nc.gpsimd.dma_start(d3[0:C, 0:1, 1:W + 1], hal[0, :, hrow_a:hrow_a + 1, :])
        nc.gpsimd.dma_start(d3[C:128, 0:1, 1:W + 1], hal[0, :, hrow_b:hrow_b + 1, :])
    if rr + WIN == HH:
        nc.gpsimd.dma_start(d3[0:C, 9:10, 1:W + 1],
                            hal[0, :, hrow_a + 1:hrow_a + 2, :])
        nc.gpsimd.dma_start(d3[C:128, 9:10, 1:W + 1],
                            hal[0, :, hrow_b + 1:hrow_b + 2, :])


# contiguous runs of row-pad columns within the [0, SPAN) output window
_PAD_RUNS = [(0, 1)] + [(257 + R * k, 2) for k in range(WIN - 1)] + [(SPAN - 1, 1)]


def _body(tc, nc, x1, x2, hal, lqkT, wv9, pT, temp64, ident, mask64, y):
    mm = nc.tensor.matmul
    f = F32

    wpool = tc.alloc_tile_pool(name="weights", bufs=1)
    dram = tc.alloc_tile_pool(name="dram", bufs=1, space="DRAM")
    accp = tc.alloc_tile_pool(name="ps_acc", bufs=1, space=bass.MemorySpace.PSUM)
    persist = tc.alloc_tile_pool(name="persist", bufs=1)

    w_lqkT = wpool.tile([128, 9 * C], f)
    nc.gpsimd.dma_start(w_lqkT[:], lqkT[:])
    w_wv9 = wpool.tile([C, 9 * C], f)
    nc.gpsimd.dma_start(w_wv9[:], wv9[:])
    w_pT = wpool.tile([C, C], f)
    nc.gpsimd.dma_start(w_pT[:], pT[:])
    w_temp = wpool.tile([C, 1], f)
    nc.gpsimd.dma_start(w_temp[:], temp64[:])
    w_id = wpool.tile([128, 128], f)
    nc.gpsimd.dma_start(w_id[:], ident[:])
    w_mask = wpool.tile([C, C], f)
    nc.gpsimd.dma_start(w_mask[:], mask64[:])

    acc_ps = accp.tile([C, C], f)          # S accumulator (q.kT)
    qk2 = persist.tile([128, 1], f)        # running sum q^2 (top) / k^2 (bottom)
    nc.vector.memset(qk2[:], 0.0)

    # ---------------- pass 1: q,k conv -> transpose -> S, norms ----------
    xws = [persist.tile([128, XT], f, tag=f"xw{i}", name=f"xw{i}")
           for i in range(3)]
    qkwins = [persist.tile([128, QKW], f, tag=f"qkw{i}", name=f"qkw{i}")
              for i in range(3)]
    for t in xws + qkwins:
        nc.vector.memset(t[:], 0.0)

    sub_idx = 0
    with (
        tc.tile_pool(name="trsb", bufs=3) as trsbp,
        tc.tile_pool(name="scratch", bufs=1) as scrp,
        tc.tile_pool(name="ps_conv", bufs=2, space=bass.MemorySpace.PSUM) as pcv,
        tc.tile_pool(name="ps_tr", bufs=2, space=bass.MemorySpace.PSUM) as ptr,
    ):
        scratch = scrp.tile([128, QKW], f)
        acc_tmp = scrp.tile([128, 1], f, tag="acctmp")
        for w in range(NW):
            xw = xws[w % 3]
            _load_window(nc, xw, w, x1, x2, hal, 0, 2)

            qkwin = qkwins[w % 3]
            for lc in range(0, SPAN, 512):
                L = min(512, SPAN - lc)
                qps = pcv.tile([128, 512], f, tag="qps")
                kps = pcv.tile([128, 512], f, tag="kps")
                for t in range(9):
                    o = lc + TAP_OFF[t]
                    mm(qps[0:C, 0:L], w_lqkT[0:C, t * C:(t + 1) * C],
                       xw[0:C, o:o + L], start=(t == 0), stop=(t == 8),
                       tile_position=(0, 0))
                    mm(kps[C:128, 0:L], w_lqkT[C:128, t * C:(t + 1) * C],
                       xw[C:128, o:o + L], start=(t == 0), stop=(t == 8),
                       tile_position=(64, 64))
                nc.scalar.copy(qkwin[0:C, lc:lc + L], qps[0:C, 0:L])
                nc.scalar.copy(qkwin[C:128, lc:lc + L], kps[C:128, 0:L])

            # zero the per-row pad columns so they don't pollute S / norms
            for off, ln in _PAD_RUNS:
                nc.gpsimd.memset(qkwin[:, off:off + ln], 0.0)

            # norms: accumulate sum of squares over this window
            nc.scalar.activation(
                scratch[:], qkwin[:],
                mybir.ActivationFunctionType.Square, accum_out=acc_tmp[:])
            nc.vector.tensor_add(qk2[:], qk2[:], acc_tmp[:])

            # S += Tq.T @ Tk per 128-col sub-chunk
            for j in range(NSUB):
                trps = ptr.tile([128, 128], f, tag="trps")
                nc.tensor.transpose(trps[:], qkwin[:, j * 128:(j + 1) * 128], w_id[:])
                trsb = trsbp.tile([128, 128], f)
                nc.scalar.copy(trsb[:], trps[:])
                mm(acc_ps[:], trsb[:, 0:C], trsb[:, C:128],
                   start=(sub_idx == 0), stop=(sub_idx == NW * NSUB - 1))
                sub_idx += 1

    # ---------------- collective: S and norms over the batch pair --------
    cc_sb = persist.tile([128, C + 1], f, tag="ccsb")
    nc.vector.memset(cc_sb[:], 0.0)
    nc.scalar.copy(cc_sb[0:C, 0:C], acc_ps[:])
    nc.vector.tensor_copy(cc_sb[:, C:C + 1], qk2[:])
    cc_in = dram.tile([128, C + 1], f)
    cc_out = dram.tile([128, C + 1], f, tag="cc_out")
    nc.sync.dma_start(cc_in[:], cc_sb[:])
    nc.gpsimd.collective_compute(
        "AllReduce", mybir.AluOpType.add,
        replica_groups=[[0, 1], [2, 3], [4, 5], [6, 7]],
        ins=[cc_in.opt()], outs=[cc_out.opt()],
    )
    sqk = persist.tile([128, C + 1], f, tag="sqk")
    nc.sync.dma_start(sqk[:], cc_out[:])

    # ---------------- tiny mid-section: softmax, M^T, G^T ----------------
    with (
        tc.tile_pool(name="mid", bufs=1) as midp,
        tc.tile_pool(name="ps_mid", bufs=1, space=bass.MemorySpace.PSUM) as pmid,
    ):
        nrm = midp.tile([128, 1], f, tag="nrm")       # sqrt of sums
        nc.scalar.sqrt(nrm[:], sqk[:, C:C + 1])
        nc.vector.tensor_scalar_max(nrm[:], nrm[:], EPS)
        rn = midp.tile([128, 1], f, tag="rn")         # 1/||.||
        nc.vector.reciprocal(rn[:], nrm[:])
        rs = midp.tile([C, 1], f, tag="rs")           # temp/||q|| per row c
        nc.vector.tensor_mul(rs[:], rn[0:C, :], w_temp[:])

        # broadcast 1/||k|| along free dim: transpose then rank-1 outer
        nkT_ps = pmid.tile([1, C], f, tag="nkT")
        nc.tensor.transpose(nkT_ps[:], rn[C:128, :], w_id[C:128, C:128])
        nkT = midp.tile([1, C], f, tag="nkT_sb")
        nc.scalar.copy(nkT[:], nkT_ps[:])
        ones1 = midp.tile([1, C], f, tag="ones1")
        nc.vector.memset(ones1[:], 1.0)
        nkb_ps = pmid.tile([C, C], f, tag="nkb")
        mm(nkb_ps[:], ones1[:], nkT[:])
        # logits = S * rs(row) * (1/||k||)(col)
        sp = midp.tile([C, C], f, tag="sp")
        nc.vector.tensor_scalar(sp[:], sqk[0:C, 0:C], rs[:], None,
                                op0=mybir.AluOpType.mult)
        nc.vector.tensor_mul(sp[:], sp[:], nkb_ps[:])

        # blockwise softmax via additive off-block mask (-1e30):
        # off-block entries exp to exactly 0, so the result IS Ablk.
        nc.vector.tensor_add(sp[:], sp[:], w_mask[:])
        negm = midp.tile([C, 1], f, tag="negm")
        nc.vector.tensor_reduce(negm[:], sp[:], axis=mybir.AxisListType.X,
                                op=mybir.AluOpType.max, negate=True)
        den = midp.tile([C, 1], f, tag="den")
        ex = midp.tile([C, C], f, tag="ex")
        nc.scalar.activation(ex[:], sp[:], mybir.ActivationFunctionType.Exp,
                             bias=negm[:], scale=1.0, accum_out=den[:])
        rden = midp.tile([C, 1], f, tag="rden")
        nc.vector.reciprocal(rden[:], den[:])
        ablk = midp.tile([C, C], f, tag="ablk")
        nc.vector.tensor_scalar(ablk[:], ex[:], rden[:], None,
                                op0=mybir.AluOpType.mult)

        # M^T = Ablk^T @ P^T
        mt_ps = pmid.tile([C, C], f, tag="mt")
        mm(mt_ps[:], ablk[:], w_pT[:])
        mt = midp.tile([C, C], f, tag="mt_sb")
        nc.scalar.copy(mt[:], mt_ps[:])

        # G^T[dp] = wv9[dp] @ M^T, duplicated to partitions 64:128
        gt_ps = pmid.tile([128, 9 * C], f, tag="gt")
        for dp in range(9):
            s = dp * C
            mm(gt_ps[0:C, s:s + C], w_wv9[:, s:s + C], mt[:],
               tile_position=(0, 0))
            mm(gt_ps[C:128, s:s + C], w_wv9[:, s:s + C], mt[:],
               tile_position=(0, 64))
        gt = persist.tile([128, 9 * C], f, tag="gt_sb")
        nc.scalar.copy(gt[:], gt_ps[:])

    accp.release()

    # ---------------- pass 2: out = G-conv(x2), write y ------------------
    with (
        tc.tile_pool(name="osb", bufs=3) as osbp,
        tc.tile_pool(name="ps_p2", bufs=2, space=bass.MemorySpace.PSUM) as pp2,
    ):
        for w in range(NW):
            xw = xws[w % 3]
            _load_window(nc, xw, w, x2, x2, hal, 2, 2)
            rr = WIN * w
            oswin = qkwins[w % 3]  # reuse as [64, 8*256] output staging
            for r in range(WIN):
                o2a = pp2.tile([C, W], f, tag="o2a")
                o2b = pp2.tile([C, W], f, tag="o2b")
                for t, (dy, dx) in enumerate(TAPS):
                    o = (r + 1 + dy) * R + dx + 2
                    if t % 2 == 0:
                        mm(o2a[:], gt[0:C, t * C:(t + 1) * C],
                           xw[0:C, o:o + W], start=(t == 0), stop=(t == 8),
                           tile_position=(0, 0))
                    else:
                        mm(o2b[:], gt[C:128, t * C:(t + 1) * C],
                           xw[C:128, o:o + W], start=(t == 1), stop=(t == 7),
                           tile_position=(64, 0))
                obs = osbp.tile([C, W], f, tag="obs")
                nc.scalar.copy(obs[:], o2b[:])
                nc.vector.tensor_add(oswin[0:C, r * W:(r + 1) * W], o2a[:], obs[:])
            nc.sync.dma_start(
                y[0, :, rr:rr + WIN, :],
                oswin[0:C, 0:WIN * W].rearrange("p (r c) -> p r c", c=W))

    for p in (persist, dram, wpool):
        p.release()


# ======================= host side =========================================

def _prep_consts(q_w, q_dw_w, kv_w, kv_dw_w, proj_w, temperature):
    q_w = np.asarray(q_w, np.float32)[:, :, 0, 0]          # [o, i]
    kv_w = np.asarray(kv_w, np.float32)[:, :, 0, 0]        # [2C, i]
    q_dw = np.asarray(q_dw_w, np.float32)[:, 0]            # [C, 3, 3]
    kv_dw = np.asarray(kv_dw_w, np.float32)[:, 0]          # [2C, 3, 3]
    proj = np.asarray(proj_w, np.float32)[:, :, 0, 0]      # [o, c]
    temp = np.asarray(temperature, np.float32).reshape(HEADS)

    lqkT = np.zeros((128, 9 * C), np.float32)
    wv9 = np.zeros((C, 9 * C), np.float32)
    for t, (dy, dx) in enumerate(TAPS):
        w9q = q_dw[:, dy + 1, dx + 1][:, None] * q_w       # [o, i]
        w9k = kv_dw[0:C, dy + 1, dx + 1][:, None] * kv_w[0:C]
        lqkT[0:C, t * C:(t + 1) * C] = w9q.T
        lqkT[C:128, t * C:(t + 1) * C] = w9k.T
        # wv9[dp][d, i] = wdv[d, dp] * Wv[d, i]
        wv9[:, t * C:(t + 1) * C] = (
            kv_dw[C:2 * C, dy + 1, dx + 1][:, None] * kv_w[C:2 * C]
        )
    pTm = proj.T.copy()                                    # [c, o]
    temp64 = np.repeat(temp, HD).reshape(C, 1).astype(np.float32)
    ident = np.eye(128, dtype=np.float32)
    mask = np.full((C, C), -1e30, np.float32)
    for h in range(HEADS):
        mask[h * HD:(h + 1) * HD, h * HD:(h + 1) * HD] = 0.0
    return lqkT, wv9, pTm, temp64, ident, mask


def _build_halos(input1, input2):
    """[8, C, 4, W]: per core (b-major, h) rows: x1 top, x1 bot, x2 top,
    x2 bot one-row halos (zeros at the outer image edges)."""
    hal = np.zeros((B, 2, C, 4, W), np.float32)
    hal[:, 1, :, 0, :] = input1[:, :, HH - 1, :]
    hal[:, 0, :, 1, :] = input1[:, :, HH, :]
    hal[:, 1, :, 2, :] = input2[:, :, HH - 1, :]
    hal[:, 0, :, 3, :] = input2[:, :, HH, :]
    return hal.reshape(B * 2, C, 4, W)


def kernel(input1, input2, q_w, q_dw_w, kv_w, kv_dw_w, proj_w, temperature):
    if "runner" not in _CACHE:
        _CACHE["nc"] = build_module()
        _make_runner(_CACHE["nc"])
    run = _CACHE["runner"]

    lqkT, wv9, pTm, temp64, ident, mask = _prep_consts(
        q_w, q_dw_w, kv_w, kv_dw_w, proj_w, temperature)

    input1 = np.ascontiguousarray(np.asarray(input1, np.float32))
    input2 = np.ascontiguousarray(np.asarray(input2, np.float32))
    hal = _build_halos(input1, input2)
    out = run({"x1": input1, "x2": input2, "hal": hal, "lqkT": lqkT,
               "wv9": wv9, "pT": pTm, "temp64": temp64, "ident": ident,
               "mask64": mask})
    return np.asarray(out)


def _make_runner(nc):
    """jit(shard_map) over a ('b','h')=(4,2) mesh: raw inputs sharded
    batch x H-half, weights replicated, output assembled on device."""
    import jax
    from jax.sharding import Mesh, NamedSharding, PartitionSpec as P
    from jax.experimental.shard_map import shard_map
    from concourse import bass2jax as b2j
    from concourse import mybir as _mb

    b2j.install_neuronx_cc_hook()
    partition_name = nc.partition_id_tensor.name if nc.partition_id_tensor else None
    in_names, out_names, out_avals = [], [], []
    for alloc in nc.m.functions[0].allocations:
        if not isinstance(alloc, _mb.MemoryLocationSet):
            continue
        name = alloc.memorylocations[0].name
        if alloc.kind == "ExternalInput":
            if name != partition_name:
                in_names.append(name)
        elif alloc.kind == "ExternalOutput":
            out_names.append(name)
            shape = tuple(alloc.tensor_shape)
            dtype = _mb.dt.np(alloc.dtype)
            out_avals.append(jax.core.ShapedArray(shape, dtype))
    all_in_names = list(in_names) + list(out_names)
    if partition_name is not None:
        all_in_names.append(partition_name)

    SHARD = P("b", None, "h", None)
    spec_by_name = {
        "x1": SHARD, "x2": SHARD, "hal": P(("b", "h"), None, None, None),
        "y": SHARD,
    }
    in_specs = tuple(spec_by_name.get(nm, P()) for nm in in_names + out_names)

    devices = jax.devices()[:8]
    mesh = Mesh(np.asarray(devices).reshape(B, 2), ("b", "h"))

    def _pjrt_body(*args):
        operands = list(args)
        if partition_name is not None:
            operands.append(b2j.partition_id_tensor())
        return tuple(b2j._bass_exec_p.bind(
            *operands, out_avals=tuple(out_avals), in_names=tuple(all_in_names),
            out_names=tuple(out_names), lowering_input_output_aliases=(),
            sim_require_finite=True, sim_require_nnan=True, nc=nc))

    sharded = jax.jit(
        shard_map(_pjrt_body, mesh=mesh, in_specs=in_specs,
                  out_specs=(SHARD,), check_rep=False),
        keep_unused=True)

    # persistent dummy for the y operand (content ignored: the NEFF's y
    # tensor is bound to the result buffer, the operand only fixes order)
    ydummy = jax.device_put(
        np.zeros((B, C, H, W), np.float32), NamedSharding(mesh, SHARD))

    def run(in_map):
        args = [in_map[nm] for nm in in_names] + [ydummy]
        return sharded(*args)[0]

    _CACHE["runner"] = run
    return run


# revision 26
# speedup vs baseline: 16754.6525x; 14159.3268x over previous
"""CAB (channel-attention block) Trainium2 kernel.

Sharding: 8 cores = 4 batches x 2 H-halves (mesh ('b','h') = (4,2)).
Inputs are passed RAW ([4,C,256,256] f32) and sharded by jax directly onto
the cores (no host-side padding/copies). One-row halos come from a tiny
host-built [8,C,4,W] tensor.

Device kernel (per core, [C,128,256] slice):
  - x1|x2 are DMA-cast (f32->bf16) into 4 persistent padded SBUF slab
    quarters (34 rows each incl 1-row halos, 258-col padded rows, pads
    pre-zeroed); pass 2 reads x2 from a half-height slab + the x2 half of
    the pass-1 slab.
  - pass 1: 9-tap folded q,k conv as block-diagonal [128x128] bf16 matmuls
    (q and k in one K=128 chain), psum -> bf16 qkwin, DMA-engine
    transposes (XBAR) per 128-pixel chunk, S += Tq.T @ Tk on the PE;
    norm sums via Square-activation accumulate.
  - AllReduce (S, |q|^2, |k|^2) across the 2 cores sharing a batch.
  - tiny softmax / fold: G[dp] = (P @ A_blkdiag) @ (diag(wdv[:,dp]) @ Wv).
  - pass 2: out = G-conv(x2) as two concurrent 4/5-tap PE quadrant chains.

Math folds used (all exact):
  - L2 normalize folds into S: attn_logits = S * temp / (||q|| ||k||^T).
  - proj o (attn @ v) o dwconv_v o pwconv_v folds into the data-dependent
    G 3x3 dense conv on input2.
"""
import sys

sys.path.insert(0, "/opt/trn_rl_repo")

import numpy as np

import concourse.bacc as bacc
import concourse.bass as bass
import concourse.tile as tile
from concourse import mybir

F32 = mybir.dt.float32
BF16 = mybir.dt.bfloat16

B, C, H, W = 4, 64, 256, 256
HEADS = 8
HD = C // HEADS
EPS = 1e-12

HH = H // 2            # rows per core
R = W + 2              # padded row length
WIN = 8                # output rows per window
NW = HH // WIN         # windows per pass
SPAN = WIN * R         # 2064 output positions per window (incl row pads)
QKW = 2176             # qkwin width: SPAN rounded up to 17*128 (tail zeroed)
NSUB = QKW // 128      # S sub-chunks per window
QROWS = 34             # slab quarter rows (32 data + 2 halo)
QT = 1 + QROWS * R + 1  # slab tile: guard col + 34 padded rows + spare

TAPS = [(dy, dx) for dy in (-1, 0, 1) for dx in (-1, 0, 1)]
TAPREL = [dy * R + dx for dy, dx in TAPS]

_CACHE = {}


def build_module():
    nc = bacc.Bacc("TRN2", target_bir_lowering=False, debug=False, num_devices=8)

    x1 = nc.declare_dram_parameter("x1", [1, C, HH, W], F32, isOutput=False)
    x2 = nc.declare_dram_parameter("x2", [1, C, HH, W], F32, isOutput=False)
    hal = nc.declare_dram_parameter("hal", [1, C, 4, W], F32, isOutput=False)
    w1 = nc.declare_dram_parameter("w1", [128, 9 * 128], F32, isOutput=False)
    wv9 = nc.declare_dram_parameter("wv9", [C, 9 * C], F32, isOutput=False)
    pT = nc.declare_dram_parameter("pT", [C, C], F32, isOutput=False)
    temp64 = nc.declare_dram_parameter("temp64", [C, 1], F32, isOutput=False)
    ident = nc.declare_dram_parameter("ident", [128, 128], F32, isOutput=False)
    mask64 = nc.declare_dram_parameter("mask64", [C, C], F32, isOutput=False)
    y = nc.declare_dram_parameter("y", [1, C, HH, W], F32, isOutput=True)

    with tile.TileContext(nc) as tc:
        _body(tc, nc, x1, x2, hal, w1, wv9, pT, temp64, ident, mask64, y)
    nc.compile()
    return nc


def _slab3(slab):
    return slab[:, 1:1 + QROWS * R].rearrange("p (r c) -> p r c", c=R)


def _body(tc, nc, x1, x2, hal, w1, wv9, pT, temp64, ident, mask64, y):
    mm = nc.tensor.matmul
    f = F32

    wpool = tc.alloc_tile_pool(name="weights", bufs=1)
    dram = tc.alloc_tile_pool(name="dram", bufs=1, space="DRAM")
    accp = tc.alloc_tile_pool(name="ps_acc", bufs=1, space=bass.MemorySpace.PSUM)
    persist = tc.alloc_tile_pool(name="persist", bufs=1)

    w1sb = wpool.tile([128, 9 * 128], BF16)
    nc.gpsimd.dma_start(w1sb[:], w1[:])
    w_wv9 = wpool.tile([C, 9 * C], f)
    nc.gpsimd.dma_start(w_wv9[:], wv9[:])
    w_pT = wpool.tile([C, C], f)
    nc.gpsimd.dma_start(w_pT[:], pT[:])
    w_temp = wpool.tile([C, 1], f)
    nc.gpsimd.dma_start(w_temp[:], temp64[:])
    w_id = wpool.tile([128, 128], f)
    nc.gpsimd.dma_start(w_id[:], ident[:])
    w_mask = wpool.tile([C, C], f)
    nc.gpsimd.dma_start(w_mask[:], mask64[:])

    # Gram accumulators [[q.qT, q.kT], [k.qT, k.kT]]: S plus both norm
    # diagonals; two halves so the first AllReduce overlaps pass 1
    acc_a = accp.tile([128, 128], f, tag="acc_a")
    acc_b = accp.tile([128, 128], f, tag="acc_b")

    # ------- persistent bf16 slabs: s1 = x1|x2 (pass 1), s2 = x2 (pass 2) ----
    s1 = [persist.tile([128, QT], BF16, tag=f"s1_{q}", name=f"s1_{q}")
          for q in range(4)]
    s2 = [persist.tile([C, QT], BF16, tag=f"s2_{q}", name=f"s2_{q}")
          for q in range(4)]
    for t in s1 + s2:
        # only the pad columns need zeroing: guard+spare cols and the
        # left/right pad column of each padded row (strided memsets)
        t3 = t[:, 1:1 + QROWS * R].rearrange("p (r c) -> p r c", c=R)
        nc.vector.memset(t3[:, :, 0:1], 0.0)
        nc.vector.memset(t3[:, :, R - 1:R], 0.0)
        nc.vector.memset(t[:, 0:1], 0.0)
        nc.vector.memset(t[:, QT - 1:QT], 0.0)
    def load_pieces(dst3, part, src, qd, nrows=9):
        """closures DMA-ing ~nrows-row pieces of a slab quarter"""
        g0, g1 = max(0, 32 * qd - 1), min(HH, 32 * qd + 33)
        l0 = g0 - (32 * qd - 1)
        out = []
        for a in range(0, g1 - g0, nrows):
            b = min(a + nrows, g1 - g0)
            out.append(lambda a=a, b=b: nc.gpsimd.dma_start(
                dst3[part, l0 + a:l0 + b, 1:W + 1], src[0, :, g0 + a:g0 + b, :]))
        return out

    def halo_pieces(dst3, part, hrow, lr):
        return [lambda: nc.gpsimd.dma_start(
            dst3[part, lr:lr + 1, 1:W + 1], hal[0, :, hrow:hrow + 1, :])]

    TOP, BOT = slice(0, C), slice(C, 128)
    s1p = []
    for qd in range(4):
        d1 = _slab3(s1[qd])
        p = load_pieces(d1, TOP, x1, qd) + load_pieces(d1, BOT, x2, qd)
        if qd == 0:
            p += halo_pieces(d1, TOP, 0, 0) + halo_pieces(d1, BOT, 2, 0)
        if qd == 3:
            p += halo_pieces(d1, TOP, 1, 33) + halo_pieces(d1, BOT, 3, 33)
        s1p.append(p)
    s2p = []
    for qd in range(4):
        d2 = _slab3(s2[qd])
        p = load_pieces(d2, TOP, x2, qd)
        if qd == 0:
            p += halo_pieces(d2, TOP, 2, 0)
        if qd == 3:
            p += halo_pieces(d2, TOP, 3, 33)
        s2p.append(p)

    # quarter 0+1 fully upfront, rest drip-fed between windows
    for fn in s1p[0] + s1p[1]:
        fn()
    pending = s1p[2] + s1p[3] + s2p[0] + s2p[1] + s2p[2] + s2p[3]

    # contiguous runs of row-pad columns within the [0, SPAN) output window
    pad_runs = [(0, 1)] + [(257 + R * k, 2) for k in range(WIN - 1)] \
        + [(SPAN - 1, 1)]

    # ---------------- pass 1: q,k conv -> XBAR transpose -> S, norms -------
    qkwins = [persist.tile([128, QKW], BF16, tag=f"qkw{i}", name=f"qkw{i}")
              for i in range(4)]
    trwins = [persist.tile([128, QKW], BF16, tag=f"trw{i}", name=f"trw{i}")
              for i in range(4)]
    for t in qkwins:
        # zero the tail [SPAN:QKW] (rest is overwritten every window)
        nc.vector.memset(t[:, SPAN:QKW], 0.0)

    cc_sb_a = persist.tile([128, 128], f, tag="ccsb_a")
    cc_in_a = dram.tile([128, 128], f, tag="cc_in_a")
    cc_out_a = dram.tile([128, 128], f, tag="cc_out_a")

    sub_idx = 0
    with (
        tc.tile_pool(name="ps_conv", bufs=4, space=bass.MemorySpace.PSUM) as pcv,
    ):
        HALF = NW * NSUB // 2

        def emit_gram(w):
            nonlocal sub_idx
            trw = trwins[w % 4]
            acc = acc_a if w < NW // 2 else acc_b
            for j in range(NSUB):
                mm(acc[:], trw[:, j * 128:(j + 1) * 128],
                   trw[:, j * 128:(j + 1) * 128],
                   start=(sub_idx % HALF == 0), stop=(sub_idx % HALF == HALF - 1))
                sub_idx += 1

        for w in range(NW):
            qd, wl = w // 4, w % 4
            slab = s1[qd]
            wbase = 1 + (8 * wl + 1) * R   # slab pos of output j=0
            qkwin = qkwins[w % 4]
            for lc in range(0, SPAN, 512):
                L = min(512, SPAN - lc)
                ps = pcv.tile([128, 512], f, tag="qkps")
                for t in range(9):
                    o = wbase + lc + TAPREL[t]
                    mm(ps[:, 0:L], w1sb[:, t * 128:(t + 1) * 128],
                       slab[:, o:o + L], start=(t == 0), stop=(t == 8))
                nc.scalar.copy(qkwin[:, lc:lc + L], ps[:, 0:L])

            # zero the per-row pad columns so they don't pollute S / norms
            for off, ln in pad_runs:
                nc.vector.memset(qkwin[:, off:off + ln], 0.0)

            # all NSUB 128-col transposes in one batched XBAR instruction
            nc.sync.dma_start_transpose(
                trwins[w % 4][:].rearrange("p (j c) -> p j c", c=128), qkwin[:])

            # Gram += T.T @ T, emitted two windows late so the PE's
            # in-order queue never stalls on the XBAR transpose
            if w >= 2:
                emit_gram(w - 2)
            # just-in-time slab loads, keeping early DMA contention low
            if w in (2, 5):
                for fn in (s1p[2] if w == 2 else s1p[3]):
                    fn()
            if w in (7, 9, 11, 13):
                for fn in s2p[(w - 7) // 2]:
                    fn()
            if w == NW // 2 - 1:
                # first-half AllReduce overlaps the second half of pass 1
                nc.scalar.copy(cc_sb_a[:], acc_a[:])
                nc.sync.dma_start(cc_in_a[:], cc_sb_a[:])
                nc.gpsimd.collective_compute(
                    "AllReduce", mybir.AluOpType.add,
                    replica_groups=[[0, 1], [2, 3], [4, 5], [6, 7]],
                    ins=[cc_in_a.opt()], outs=[cc_out_a.opt()],
                )
        emit_gram(NW - 2)
        emit_gram(NW - 1)

    # ---------------- collective: second Gram half, then combine ---------
    cc_sb_b = persist.tile([128, 128], f, tag="ccsb_b")
    nc.scalar.copy(cc_sb_b[:], acc_b[:])
    cc_in_b = dram.tile([128, 128], f, tag="cc_in_b")
    cc_out_b = dram.tile([128, 128], f, tag="cc_out_b")
    nc.sync.dma_start(cc_in_b[:], cc_sb_b[:])
    nc.gpsimd.collective_compute(
        "AllReduce", mybir.AluOpType.add,
        replica_groups=[[0, 1], [2, 3], [4, 5], [6, 7]],
        ins=[cc_in_b.opt()], outs=[cc_out_b.opt()],
    )
    sqk_a = persist.tile([128, 128], f, tag="sqk_a")
    nc.sync.dma_start(sqk_a[:], cc_out_a[:])
    sqk_b = persist.tile([128, 128], f, tag="sqk_b")
    nc.sync.dma_start(sqk_b[:], cc_out_b[:])
    sqk = persist.tile([128, 128], f, tag="sqk")
    nc.vector.tensor_add(sqk[:], sqk_a[:], sqk_b[:])

    # ---------------- tiny mid-section: softmax, M^T, G^T ----------------
    with (
        tc.tile_pool(name="mid", bufs=1) as midp,
        tc.tile_pool(name="ps_mid", bufs=1, space=bass.MemorySpace.PSUM) as pmid,
    ):
        # norm sums = diag(Gram): mask with identity, reduce along free
        dgm = midp.tile([128, 128], f, tag="dgm")
        nc.vector.tensor_mul(dgm[:], sqk[:], w_id[:])
        qk2 = midp.tile([128, 1], f, tag="qk2")
        nc.vector.tensor_reduce(qk2[:], dgm[:], axis=mybir.AxisListType.X,
                                op=mybir.AluOpType.add)
        nrm = midp.tile([128, 1], f, tag="nrm")       # sqrt of sums
        nc.scalar.sqrt(nrm[:], qk2[:])
        nc.vector.tensor_scalar_max(nrm[:], nrm[:], EPS)
        rn = midp.tile([128, 1], f, tag="rn")         # 1/||.||
        nc.vector.reciprocal(rn[:], nrm[:])
        rs = midp.tile([C, 1], f, tag="rs")           # temp/||q|| per row c
        nc.vector.tensor_mul(rs[:], rn[0:C, :], w_temp[:])

        # broadcast 1/||k|| along free dim: transpose then rank-1 outer
        nkT_ps = pmid.tile([1, C], f, tag="nkT")
        nc.tensor.transpose(nkT_ps[:], rn[C:128, :], w_id[C:128, C:128])
        nkT = midp.tile([1, C], f, tag="nkT_sb")
        nc.scalar.copy(nkT[:], nkT_ps[:])
        ones1 = midp.tile([1, C], f, tag="ones1")
        nc.vector.memset(ones1[:], 1.0)
        nkb_ps = pmid.tile([C, C], f, tag="nkb")
        mm(nkb_ps[:], ones1[:], nkT[:])
        # logits = S * rs(row) * (1/||k||)(col); S = q.kT block of the Gram
        sp = midp.tile([C, C], f, tag="sp")
        nc.vector.tensor_scalar(sp[:], sqk[0:C, C:128], rs[:], None,
                                op0=mybir.AluOpType.mult)
        nc.vector.tensor_mul(sp[:], sp[:], nkb_ps[:])

        # blockwise softmax via additive off-block mask (-1e30):
        # off-block entries exp to exactly 0, so the result IS Ablk.
        nc.vector.tensor_add(sp[:], sp[:], w_mask[:])
        negm = midp.tile([C, 1], f, tag="negm")
        nc.vector.tensor_reduce(negm[:], sp[:], axis=mybir.AxisListType.X,
                                op=mybir.AluOpType.max, negate=True)
        den = midp.tile([C, 1], f, tag="den")
        ex = midp.tile([C, C], f, tag="ex")
        nc.scalar.activation(ex[:], sp[:], mybir.ActivationFunctionType.Exp,
                             bias=negm[:], scale=1.0, accum_out=den[:])
        rden = midp.tile([C, 1], f, tag="rden")
        nc.vector.reciprocal(rden[:], den[:])
        ablk = midp.tile([C, C], f, tag="ablk")
        nc.vector.tensor_scalar(ablk[:], ex[:], rden[:], None,
                                op0=mybir.AluOpType.mult)

        # M^T = Ablk^T @ P^T
        mt_ps = pmid.tile([C, C], f, tag="mt")
        mm(mt_ps[:], ablk[:], w_pT[:])
        mt = midp.tile([C, C], f, tag="mt_sb")
        nc.scalar.copy(mt[:], mt_ps[:])

        # G^T[dp] = wv9[dp] @ M^T, duplicated to partitions 64:128 (bf16)
        gt_ps = pmid.tile([128, 9 * C], f, tag="gt")
        for dp in range(9):
            s = dp * C
            mm(gt_ps[0:C, s:s + C], w_wv9[:, s:s + C], mt[:],
               tile_position=(0, 0))
            mm(gt_ps[C:128, s:s + C], w_wv9[:, s:s + C], mt[:],
               tile_position=(0, 64))
        gt = persist.tile([128, 9 * C], BF16, tag="gt_sb")
        nc.scalar.copy(gt[:], gt_ps[:])

    accp.release()

    # ---------------- pass 2: out = G-conv(x2), write y ------------------
    # chain A (even taps) reads x2 from s2 (partitions 0:64);
    # chain B (odd taps) reads x2 from s1's upper half (partitions 64:128).
    with (
        tc.tile_pool(name="osb", bufs=2) as osbp,
        tc.tile_pool(name="ps_p2", bufs=2, space=bass.MemorySpace.PSUM) as pp2,
    ):
        for w in range(NW):
            qd, wl = w // 4, w % 4
            wbase = 1 + (8 * wl + 1) * R
            osb = osbp.tile([C, SPAN], f)
            for lc in range(0, SPAN, 512):
                L = min(512, SPAN - lc)
                o2a = pp2.tile([C, 512], f, tag="o2a")
                o2b = pp2.tile([C, 512], f, tag="o2b")
                for t in range(9):
                    o = wbase + lc + TAPREL[t]
                    if t % 2 == 0:
                        mm(o2a[:, 0:L], gt[0:C, t * C:(t + 1) * C],
                           s2[qd][:, o:o + L], start=(t == 0), stop=(t == 8),
                           tile_position=(0, 0))
                    else:
                        mm(o2b[:, 0:L], gt[C:128, t * C:(t + 1) * C],
                           s1[qd][C:128, o:o + L], start=(t == 1), stop=(t == 7),
                           tile_position=(64, 0))
                obs = osbp.tile([C, 512], f, tag="obs")
                nc.vector.tensor_copy(obs[:, 0:L], o2b[:, 0:L])
                nc.vector.tensor_add(osb[:, lc:lc + L], o2a[:, 0:L], obs[:, 0:L])
            rr = WIN * w
            nc.sync.dma_start(
                y[0, :, rr:rr + WIN, :],
                osb[:].rearrange("p (r c) -> p r c", c=R)[:, :, 1:W + 1])

    for p in (persist, dram, wpool):
        p.release()


# ======================= host side =========================================

def _prep_consts(q_w, q_dw_w, kv_w, kv_dw_w, proj_w, temperature):
    q_w = np.asarray(q_w, np.float32)[:, :, 0, 0]          # [o, i]
    kv_w = np.asarray(kv_w, np.float32)[:, :, 0, 0]        # [2C, i]
    q_dw = np.asarray(q_dw_w, np.float32)[:, 0]            # [C, 3, 3]
    kv_dw = np.asarray(kv_dw_w, np.float32)[:, 0]          # [2C, 3, 3]
    proj = np.asarray(proj_w, np.float32)[:, :, 0, 0]      # [o, c]
    temp = np.asarray(temperature, np.float32).reshape(HEADS)

    w1 = np.zeros((128, 9 * 128), np.float32)
    wv9 = np.zeros((C, 9 * C), np.float32)
    for t, (dy, dx) in enumerate(TAPS):
        w9q = q_dw[:, dy + 1, dx + 1][:, None] * q_w       # [o, i]
        w9k = kv_dw[0:C, dy + 1, dx + 1][:, None] * kv_w[0:C]
        w1[0:C, t * 128:t * 128 + C] = w9q.T
        w1[C:128, t * 128 + C:(t + 1) * 128] = w9k.T
        # wv9[dp][d, i] = wdv[d, dp] * Wv[d, i]
        wv9[:, t * C:(t + 1) * C] = (
            kv_dw[C:2 * C, dy + 1, dx + 1][:, None] * kv_w[C:2 * C]
        )
    pTm = proj.T.copy()                                    # [c, o]
    temp64 = np.repeat(temp, HD).reshape(C, 1).astype(np.float32)
    ident = np.eye(128, dtype=np.float32)
    mask = np.full((C, C), -1e30, np.float32)
    for h in range(HEADS):
        mask[h * HD:(h + 1) * HD, h * HD:(h + 1) * HD] = 0.0
    return w1, wv9, pTm, temp64, ident, mask


def _build_halos(input1, input2):
    """[8, C, 4, W]: per core (b-major, h) rows: x1 top, x1 bot, x2 top,
    x2 bot one-row halos (zeros at the outer image edges)."""
    hal = np.zeros((B, 2, C, 4, W), np.float32)
    hal[:, 1, :, 0, :] = input1[:, :, HH - 1, :]
    hal[:, 0, :, 1, :] = input1[:, :, HH, :]
    hal[:, 1, :, 2, :] = input2[:, :, HH - 1, :]
    hal[:, 0, :, 3, :] = input2[:, :, HH, :]
    return hal.reshape(B * 2, C, 4, W)


def kernel(input1, input2, q_w, q_dw_w, kv_w, kv_dw_w, proj_w, temperature):
    if "runner" not in _CACHE:
        _CACHE["nc"] = build_module()
        _make_runner(_CACHE["nc"])
    run = _CACHE["runner"]

    w1, wv9, pTm, temp64, ident, mask = _prep_consts(
        q_w, q_dw_w, kv_w, kv_dw_w, proj_w, temperature)

    input1 = np.ascontiguousarray(np.asarray(input1, np.float32))
    input2 = np.ascontiguousarray(np.asarray(input2, np.float32))
    hal = _build_halos(input1, input2)
    out = run({"x1": input1, "x2": input2, "hal": hal, "w1": w1,
               "wv9": wv9, "pT": pTm, "temp64": temp64, "ident": ident,
               "mask64": mask})
    return np.asarray(out)


def _make_runner(nc):
    """jit(shard_map) over a ('b','h')=(4,2) mesh: raw inputs sharded
    batch x H-half, weights replicated, output assembled on device."""
    import jax
    from jax.sharding import Mesh, NamedSharding, PartitionSpec as P
    from jax.experimental.shard_map import shard_map
    from concourse import bass2jax as b2j
    from concourse import mybir as _mb

    b2j.install_neuronx_cc_hook()
    partition_name = nc.partition_id_tensor.name if nc.partition_id_tensor else None
    in_names, out_names, out_avals = [], [], []
    for alloc in nc.m.functions[0].allocations:
        if not isinstance(alloc, _mb.MemoryLocationSet):
            continue
        name = alloc.memorylocations[0].name
        if alloc.kind == "ExternalInput":
            if name != partition_name:
                in_names.append(name)
        elif alloc.kind == "ExternalOutput":
            out_names.append(name)
            shape = tuple(alloc.tensor_shape)
            dtype = _mb.dt.np(alloc.dtype)
            out_avals.append(jax.core.ShapedArray(shape, dtype))
    all_in_names = list(in_names) + list(out_names)
    if partition_name is not None:
        all_in_names.append(partition_name)

    SHARD = P("b", None, "h", None)
    spec_by_name = {
        "x1": SHARD, "x2": SHARD, "hal": P(("b", "h"), None, None, None),
        "y": SHARD,
    }
    in_specs = tuple(spec_by_name.get(nm, P()) for nm in in_names + out_names)

    devices = jax.devices()[:8]
    mesh = Mesh(np.asarray(devices).reshape(B, 2), ("b", "h"))

    def _pjrt_body(*args):
        operands = list(args)
        if partition_name is not None:
            operands.append(b2j.partition_id_tensor())
        return tuple(b2j._bass_exec_p.bind(
            *operands, out_avals=tuple(out_avals), in_names=tuple(all_in_names),
            out_names=tuple(out_names), lowering_input_output_aliases=(),
            sim_require_finite=True, sim_require_nnan=True, nc=nc))

    sharded = jax.jit(
        shard_map(_pjrt_body, mesh=mesh, in_specs=in_specs,
                  out_specs=(SHARD,), check_rep=False),
        keep_unused=True)

    # persistent dummy for the y operand (content ignored: the NEFF's y
    # tensor is bound to the result buffer, the operand only fixes order)
    ydummy = jax.device_put(
        np.zeros((B, C, H, W), np.float32), NamedSharding(mesh, SHARD))

    def run(in_map):
        args = [in_map[nm] for nm in in_names] + [ydummy]
        return sharded(*args)[0]

    _CACHE["runner"] = run
    return run


# revision 30
# speedup vs baseline: 16972.9505x; 1.0130x over previous
"""CAB (channel-attention block) Trainium2 kernel.

Sharding: 8 cores = 4 batches x 2 H-halves (mesh ('b','h') = (4,2)).
Inputs are passed RAW ([4,C,256,256] f32) and sharded by jax directly onto
the cores (no host-side padding/copies). One-row halos come from a tiny
host-built [8,C,4,W] tensor.

Device kernel (per core, [C,128,256] slice):
  - x1|x2 are DMA-cast (f32->bf16) into 4 persistent padded SBUF slab
    quarters (34 rows each incl 1-row halos, 258-col padded rows, pads
    pre-zeroed); pass 2 reads x2 from a half-height slab + the x2 half of
    the pass-1 slab.
  - pass 1: 9-tap folded q,k conv as block-diagonal [128x128] bf16 matmuls
    (q and k in one K=128 chain), psum -> bf16 qkwin, DMA-engine
    transposes (XBAR) per 128-pixel chunk, S += Tq.T @ Tk on the PE;
    norm sums via Square-activation accumulate.
  - AllReduce (S, |q|^2, |k|^2) across the 2 cores sharing a batch.
  - tiny softmax / fold: G[dp] = (P @ A_blkdiag) @ (diag(wdv[:,dp]) @ Wv).
  - pass 2: out = G-conv(x2) as two concurrent 4/5-tap PE quadrant chains.

Math folds used (all exact):
  - L2 normalize folds into S: attn_logits = S * temp / (||q|| ||k||^T).
  - proj o (attn @ v) o dwconv_v o pwconv_v folds into the data-dependent
    G 3x3 dense conv on input2.
"""
import sys

sys.path.insert(0, "/opt/trn_rl_repo")

import numpy as np

import concourse.bacc as bacc
import concourse.bass as bass
import concourse.tile as tile
from concourse import mybir

F32 = mybir.dt.float32
BF16 = mybir.dt.bfloat16

B, C, H, W = 4, 64, 256, 256
HEADS = 8
HD = C // HEADS
EPS = 1e-12

HH = H // 2            # rows per core
R = W + 2              # padded row length
WIN = 8                # output rows per window
NW = HH // WIN         # windows per pass
SPAN = WIN * R         # 2064 output positions per window (incl row pads)
QKW = 2176             # qkwin width: SPAN rounded up to 17*128 (tail zeroed)
NSUB = QKW // 128      # S sub-chunks per window
QROWS = 34             # s2 slab quarter rows (32 data + 2 halo)
QT = 1 + QROWS * R + 1  # s2 slab tile: guard col + 34 padded rows + spare
EROWS = 18             # s1 slab eighth rows (16 data + 2 halo)
ET = 1 + EROWS * R + 1  # s1 slab tile: guard col + 18 padded rows + spare

TAPS = [(dy, dx) for dy in (-1, 0, 1) for dx in (-1, 0, 1)]
TAPREL = [dy * R + dx for dy, dx in TAPS]

_CACHE = {}


def build_module():
    nc = bacc.Bacc("TRN2", target_bir_lowering=False, debug=False, num_devices=8)

    x1 = nc.declare_dram_parameter("x1", [1, C, HH, W], F32, isOutput=False)
    x2 = nc.declare_dram_parameter("x2", [1, C, HH, W], F32, isOutput=False)
    hal = nc.declare_dram_parameter("hal", [1, C, 4, W], F32, isOutput=False)
    w1 = nc.declare_dram_parameter("w1", [128, 9 * 128], F32, isOutput=False)
    wv9 = nc.declare_dram_parameter("wv9", [C, 9 * C], F32, isOutput=False)
    pT = nc.declare_dram_parameter("pT", [C, C], F32, isOutput=False)
    temp64 = nc.declare_dram_parameter("temp64", [C, 1], F32, isOutput=False)
    ident = nc.declare_dram_parameter("ident", [128, 128], F32, isOutput=False)
    mask64 = nc.declare_dram_parameter("mask64", [C, C], F32, isOutput=False)
    y = nc.declare_dram_parameter("y", [1, C, HH, W], F32, isOutput=True)

    with tile.TileContext(nc) as tc:
        _body(tc, nc, x1, x2, hal, w1, wv9, pT, temp64, ident, mask64, y)
    nc.compile()
    return nc


def _slab3(slab, nrows=QROWS):
    return slab[:, 1:1 + nrows * R].rearrange("p (r c) -> p r c", c=R)


def _body(tc, nc, x1, x2, hal, w1, wv9, pT, temp64, ident, mask64, y):
    mm = nc.tensor.matmul
    f = F32

    wpool = tc.alloc_tile_pool(name="weights", bufs=1)
    dram = tc.alloc_tile_pool(name="dram", bufs=1, space="DRAM")
    accp = tc.alloc_tile_pool(name="ps_acc", bufs=1, space=bass.MemorySpace.PSUM)
    persist = tc.alloc_tile_pool(name="persist", bufs=1)

    w1sb = wpool.tile([128, 9 * 128], BF16)
    w_wv9 = wpool.tile([C, 9 * C], f)
    w_pT = wpool.tile([C, C], f)
    w_temp = wpool.tile([C, 1], f)
    w_id = wpool.tile([128, 128], f)
    w_mask = wpool.tile([C, C], f)

    # Gram accumulators [[q.qT, q.kT], [k.qT, k.kT]]: S plus both norm
    # diagonals; two halves so the first AllReduce overlaps pass 1
    acc_a = accp.tile([128, 128], f, tag="acc_a")
    acc_b = accp.tile([128, 128], f, tag="acc_b")

    # ------- persistent bf16 slabs: s1 = x1|x2 (pass 1), s2 = x2 (pass 2) ----
    s1 = [persist.tile([128, ET], BF16, tag=f"s1_{q}", name=f"s1_{q}")
          for q in range(8)]
    s2 = [persist.tile([C, QT], BF16, tag=f"s2_{q}", name=f"s2_{q}")
          for q in range(4)]
    for t, nr, tot in [(t, EROWS, ET) for t in s1] + \
                      [(t, QROWS, QT) for t in s2]:
        # only the pad columns need zeroing: guard+spare cols and the
        # left/right pad column of each padded row (strided memsets)
        t3 = _slab3(t, nr)
        nc.vector.memset(t3[:, :, 0:1], 0.0)
        nc.vector.memset(t3[:, :, R - 1:R], 0.0)
        nc.vector.memset(t[:, 0:1], 0.0)
        nc.vector.memset(t[:, tot - 1:tot], 0.0)
    TOP, BOT = slice(0, C), slice(C, 128)

    def load_block(dst3, part, src, g0, g1, l0):
        nc.gpsimd.dma_start(dst3[part, l0:l0 + g1 - g0, 1:W + 1],
                            src[0, :, g0:g1, :])

    def load_halo(dst3, part, hrow, lr):
        nc.gpsimd.dma_start(dst3[part, lr:lr + 1, 1:W + 1],
                            hal[0, :, hrow:hrow + 1, :])

    def load_s1(e):
        g0, g1 = max(0, 16 * e - 1), min(HH, 16 * e + 17)
        l0 = g0 - (16 * e - 1)
        d1 = _slab3(s1[e], EROWS)
        load_block(d1, TOP, x1, g0, g1, l0)
        load_block(d1, BOT, x2, g0, g1, l0)
        if e == 0:
            load_halo(d1, TOP, 0, 0)
            load_halo(d1, BOT, 2, 0)
        if e == 7:
            load_halo(d1, TOP, 1, 17)
            load_halo(d1, BOT, 3, 17)

    def load_s2(qd):
        g0, g1 = max(0, 32 * qd - 1), min(HH, 32 * qd + 33)
        l0 = g0 - (32 * qd - 1)
        d2 = _slab3(s2[qd])
        load_block(d2, TOP, x2, g0, g1, l0)
        if qd == 0:
            load_halo(d2, TOP, 2, 0)
        if qd == 3:
            load_halo(d2, TOP, 3, 33)

    # first eighth upfront (window 0 starts after ~2.3MB), second right
    # after; weights are tiny and go behind them on the queue
    load_s1(0)
    load_s1(1)
    nc.gpsimd.dma_start(w1sb[:], w1[:])
    nc.gpsimd.dma_start(w_wv9[:], wv9[:])
    nc.gpsimd.dma_start(w_pT[:], pT[:])
    nc.gpsimd.dma_start(w_temp[:], temp64[:])
    nc.gpsimd.dma_start(w_id[:], ident[:])
    nc.gpsimd.dma_start(w_mask[:], mask64[:])

    # contiguous runs of row-pad columns within the [0, SPAN) output window
    pad_runs = [(0, 1)] + [(257 + R * k, 2) for k in range(WIN - 1)] \
        + [(SPAN - 1, 1)]

    # ---------------- pass 1: q,k conv -> XBAR transpose -> S, norms -------
    qkwins = [persist.tile([128, QKW], BF16, tag=f"qkw{i}", name=f"qkw{i}")
              for i in range(4)]
    trwins = [persist.tile([128, QKW], BF16, tag=f"trw{i}", name=f"trw{i}")
              for i in range(4)]
    for t in qkwins:
        # zero the tail [SPAN:QKW] (rest is overwritten every window)
        nc.vector.memset(t[:, SPAN:QKW], 0.0)

    cc_sb_a = persist.tile([128, 128], f, tag="ccsb_a")
    cc_in_a = dram.tile([128, 128], f, tag="cc_in_a")
    cc_out_a = dram.tile([128, 128], f, tag="cc_out_a")

    sub_idx = 0
    with (
        tc.tile_pool(name="ps_conv", bufs=4, space=bass.MemorySpace.PSUM) as pcv,
    ):
        HALF = NW * NSUB // 2

        def emit_gram(w):
            nonlocal sub_idx
            trw = trwins[w % 4]
            acc = acc_a if w < NW // 2 else acc_b
            for j in range(NSUB):
                mm(acc[:], trw[:, j * 128:(j + 1) * 128],
                   trw[:, j * 128:(j + 1) * 128],
                   start=(sub_idx % HALF == 0), stop=(sub_idx % HALF == HALF - 1))
                sub_idx += 1

        for w in range(NW):
            slab = s1[w // 2]
            wbase = 1 + (8 * (w % 2) + 1) * R   # slab pos of output j=0
            qkwin = qkwins[w % 4]
            for lc in range(0, SPAN, 512):
                L = min(512, SPAN - lc)
                ps = pcv.tile([128, 512], f, tag="qkps")
                for t in range(9):
                    o = wbase + lc + TAPREL[t]
                    mm(ps[:, 0:L], w1sb[:, t * 128:(t + 1) * 128],
                       slab[:, o:o + L], start=(t == 0), stop=(t == 8))
                nc.scalar.copy(qkwin[:, lc:lc + L], ps[:, 0:L])

            # zero the per-row pad columns so they don't pollute S / norms
            for off, ln in pad_runs:
                nc.vector.memset(qkwin[:, off:off + ln], 0.0)

            # all NSUB 128-col transposes in one batched XBAR instruction
            nc.sync.dma_start_transpose(
                trwins[w % 4][:].rearrange("p (j c) -> p j c", c=128), qkwin[:])

            # Gram += T.T @ T, emitted two windows late so the PE's
            # in-order queue never stalls on the XBAR transpose
            if w >= 2:
                emit_gram(w - 2)
            # just-in-time slab loads, keeping early DMA contention low
            if w < 6:
                load_s1(w + 2)
            if w >= 12:
                load_s2(w - 12)
            if w == NW // 2 - 1:
                # first-half AllReduce overlaps the second half of pass 1
                nc.scalar.copy(cc_sb_a[:], acc_a[:])
                nc.sync.dma_start(cc_in_a[:], cc_sb_a[:])
                nc.gpsimd.collective_compute(
                    "AllReduce", mybir.AluOpType.add,
                    replica_groups=[[0, 1], [2, 3], [4, 5], [6, 7]],
                    ins=[cc_in_a.opt()], outs=[cc_out_a.opt()],
                )
        emit_gram(NW - 2)
        emit_gram(NW - 1)

    # ---------------- collective: second Gram half, then combine ---------
    cc_sb_b = persist.tile([128, 128], f, tag="ccsb_b")
    nc.scalar.copy(cc_sb_b[:], acc_b[:])
    cc_in_b = dram.tile([128, 128], f, tag="cc_in_b")
    cc_out_b = dram.tile([128, 128], f, tag="cc_out_b")
    nc.sync.dma_start(cc_in_b[:], cc_sb_b[:])
    nc.gpsimd.collective_compute(
        "AllReduce", mybir.AluOpType.add,
        replica_groups=[[0, 1], [2, 3], [4, 5], [6, 7]],
        ins=[cc_in_b.opt()], outs=[cc_out_b.opt()],
    )
    sqk_a = persist.tile([128, 128], f, tag="sqk_a")
    nc.sync.dma_start(sqk_a[:], cc_out_a[:])
    sqk_b = persist.tile([128, 128], f, tag="sqk_b")
    nc.sync.dma_start(sqk_b[:], cc_out_b[:])
    sqk = persist.tile([128, 128], f, tag="sqk")
    nc.vector.tensor_add(sqk[:], sqk_a[:], sqk_b[:])

    # ---------------- tiny mid-section: softmax, M^T, G^T ----------------
    with (
        tc.tile_pool(name="mid", bufs=1) as midp,
        tc.tile_pool(name="ps_mid", bufs=1, space=bass.MemorySpace.PSUM) as pmid,
    ):
        # norm sums = diag(Gram): mask with identity, reduce along free
        dgm = midp.tile([128, 128], f, tag="dgm")
        nc.vector.tensor_mul(dgm[:], sqk[:], w_id[:])
        qk2 = midp.tile([128, 1], f, tag="qk2")
        nc.vector.tensor_reduce(qk2[:], dgm[:], axis=mybir.AxisListType.X,
                                op=mybir.AluOpType.add)
        nrm = midp.tile([128, 1], f, tag="nrm")       # sqrt of sums
        nc.scalar.sqrt(nrm[:], qk2[:])
        nc.vector.tensor_scalar_max(nrm[:], nrm[:], EPS)
        rn = midp.tile([128, 1], f, tag="rn")         # 1/||.||
        nc.vector.reciprocal(rn[:], nrm[:])
        rs = midp.tile([C, 1], f, tag="rs")           # temp/||q|| per row c
        nc.vector.tensor_mul(rs[:], rn[0:C, :], w_temp[:])

        # broadcast 1/||k|| along free dim: transpose then rank-1 outer
        nkT_ps = pmid.tile([1, C], f, tag="nkT")
        nc.tensor.transpose(nkT_ps[:], rn[C:128, :], w_id[C:128, C:128])
        nkT = midp.tile([1, C], f, tag="nkT_sb")
        nc.scalar.copy(nkT[:], nkT_ps[:])
        ones1 = midp.tile([1, C], f, tag="ones1")
        nc.vector.memset(ones1[:], 1.0)
        nkb_ps = pmid.tile([C, C], f, tag="nkb")
        mm(nkb_ps[:], ones1[:], nkT[:])
        # logits = S * rs(row) * (1/||k||)(col); S = q.kT block of the Gram
        sp = midp.tile([C, C], f, tag="sp")
        nc.vector.tensor_scalar(sp[:], sqk[0:C, C:128], rs[:], None,
                                op0=mybir.AluOpType.mult)
        nc.vector.tensor_mul(sp[:], sp[:], nkb_ps[:])

        # blockwise softmax via additive off-block mask (-1e30):
        # off-block entries exp to exactly 0, so the result IS Ablk.
        nc.vector.tensor_add(sp[:], sp[:], w_mask[:])
        negm = midp.tile([C, 1], f, tag="negm")
        nc.vector.tensor_reduce(negm[:], sp[:], axis=mybir.AxisListType.X,
                                op=mybir.AluOpType.max, negate=True)
        den = midp.tile([C, 1], f, tag="den")
        ex = midp.tile([C, C], f, tag="ex")
        nc.scalar.activation(ex[:], sp[:], mybir.ActivationFunctionType.Exp,
                             bias=negm[:], scale=1.0, accum_out=den[:])
        rden = midp.tile([C, 1], f, tag="rden")
        nc.vector.reciprocal(rden[:], den[:])
        ablk = midp.tile([C, C], f, tag="ablk")
        nc.vector.tensor_scalar(ablk[:], ex[:], rden[:], None,
                                op0=mybir.AluOpType.mult)

        # M^T = Ablk^T @ P^T
        mt_ps = pmid.tile([C, C], f, tag="mt")
        mm(mt_ps[:], ablk[:], w_pT[:])
        mt = midp.tile([C, C], f, tag="mt_sb")
        nc.scalar.copy(mt[:], mt_ps[:])

        # G^T[dp] = wv9[dp] @ M^T, duplicated to partitions 64:128 (bf16)
        gt_ps = pmid.tile([128, 9 * C], f, tag="gt")
        for dp in range(9):
            s = dp * C
            mm(gt_ps[0:C, s:s + C], w_wv9[:, s:s + C], mt[:],
               tile_position=(0, 0))
            mm(gt_ps[C:128, s:s + C], w_wv9[:, s:s + C], mt[:],
               tile_position=(0, 64))
        gt = persist.tile([128, 9 * C], BF16, tag="gt_sb")
        nc.scalar.copy(gt[:], gt_ps[:])

    accp.release()

    # ---------------- pass 2: out = G-conv(x2), write y ------------------
    # chain A (even taps) reads x2 from s2 (partitions 0:64);
    # chain B (odd taps) reads x2 from s1's upper half (partitions 64:128).
    with (
        tc.tile_pool(name="osb", bufs=2) as osbp,
        tc.tile_pool(name="ps_p2", bufs=2, space=bass.MemorySpace.PSUM) as pp2,
    ):
        for w in range(NW):
            # chain A reads x2 from the s2 quarter slab, chain B from the
            # x2 half of the s1 eighth slab (different row bases)
            wq = 1 + (8 * (w % 4) + 1) * R
            we = 1 + (8 * (w % 2) + 1) * R
            osb = osbp.tile([C, SPAN], f)
            for lc in range(0, SPAN, 512):
                L = min(512, SPAN - lc)
                o2a = pp2.tile([C, 512], f, tag="o2a")
                o2b = pp2.tile([C, 512], f, tag="o2b")
                for t in range(9):
                    if t % 2 == 0:
                        o = wq + lc + TAPREL[t]
                        mm(o2a[:, 0:L], gt[0:C, t * C:(t + 1) * C],
                           s2[w // 4][:, o:o + L], start=(t == 0), stop=(t == 8),
                           tile_position=(0, 0))
                    else:
                        o = we + lc + TAPREL[t]
                        mm(o2b[:, 0:L], gt[C:128, t * C:(t + 1) * C],
                           s1[w // 2][C:128, o:o + L], start=(t == 1),
                           stop=(t == 7), tile_position=(64, 0))
                obs = osbp.tile([C, 512], f, tag="obs")
                nc.vector.tensor_copy(obs[:, 0:L], o2b[:, 0:L])
                nc.vector.tensor_add(osb[:, lc:lc + L], o2a[:, 0:L], obs[:, 0:L])
            rr = WIN * w
            nc.sync.dma_start(
                y[0, :, rr:rr + WIN, :],
                osb[:].rearrange("p (r c) -> p r c", c=R)[:, :, 1:W + 1])

    for p in (persist, dram, wpool):
        p.release()


# ======================= host side =========================================

def _prep_consts(q_w, q_dw_w, kv_w, kv_dw_w, proj_w, temperature):
    q_w = np.asarray(q_w, np.float32)[:, :, 0, 0]          # [o, i]
    kv_w = np.asarray(kv_w, np.float32)[:, :, 0, 0]        # [2C, i]
    q_dw = np.asarray(q_dw_w, np.float32)[:, 0]            # [C, 3, 3]
    kv_dw = np.asarray(kv_dw_w, np.float32)[:, 0]          # [2C, 3, 3]
    proj = np.asarray(proj_w, np.float32)[:, :, 0, 0]      # [o, c]
    temp = np.asarray(temperature, np.float32).reshape(HEADS)

    w1 = np.zeros((128, 9 * 128), np.float32)
    wv9 = np.zeros((C, 9 * C), np.float32)
    for t, (dy, dx) in enumerate(TAPS):
        w9q = q_dw[:, dy + 1, dx + 1][:, None] * q_w       # [o, i]
        w9k = kv_dw[0:C, dy + 1, dx + 1][:, None] * kv_w[0:C]
        w1[0:C, t * 128:t * 128 + C] = w9q.T
        w1[C:128, t * 128 + C:(t + 1) * 128] = w9k.T
        # wv9[dp][d, i] = wdv[d, dp] * Wv[d, i]
        wv9[:, t * C:(t + 1) * C] = (
            kv_dw[C:2 * C, dy + 1, dx + 1][:, None] * kv_w[C:2 * C]
        )
    pTm = proj.T.copy()                                    # [c, o]
    temp64 = np.repeat(temp, HD).reshape(C, 1).astype(np.float32)
    ident = np.eye(128, dtype=np.float32)
    mask = np.full((C, C), -1e30, np.float32)
    for h in range(HEADS):
        mask[h * HD:(h + 1) * HD, h * HD:(h + 1) * HD] = 0.0
    return w1, wv9, pTm, temp64, ident, mask


def _build_halos(input1, input2):
    """[8, C, 4, W]: per core (b-major, h) rows: x1 top, x1 bot, x2 top,
    x2 bot one-row halos (zeros at the outer image edges)."""
    hal = np.zeros((B, 2, C, 4, W), np.float32)
    hal[:, 1, :, 0, :] = input1[:, :, HH - 1, :]
    hal[:, 0, :, 1, :] = input1[:, :, HH, :]
    hal[:, 1, :, 2, :] = input2[:, :, HH - 1, :]
    hal[:, 0, :, 3, :] = input2[:, :, HH, :]
    return hal.reshape(B * 2, C, 4, W)


def kernel(input1, input2, q_w, q_dw_w, kv_w, kv_dw_w, proj_w, temperature):
    if "runner" not in _CACHE:
        _CACHE["nc"] = build_module()
        _make_runner(_CACHE["nc"])
    run = _CACHE["runner"]

    w1, wv9, pTm, temp64, ident, mask = _prep_consts(
        q_w, q_dw_w, kv_w, kv_dw_w, proj_w, temperature)

    input1 = np.ascontiguousarray(np.asarray(input1, np.float32))
    input2 = np.ascontiguousarray(np.asarray(input2, np.float32))
    hal = _build_halos(input1, input2)
    out = run({"x1": input1, "x2": input2, "hal": hal, "w1": w1,
               "wv9": wv9, "pT": pTm, "temp64": temp64, "ident": ident,
               "mask64": mask})
    return np.asarray(out)


def _make_runner(nc):
    """jit(shard_map) over a ('b','h')=(4,2) mesh: raw inputs sharded
    batch x H-half, weights replicated, output assembled on device."""
    import jax
    from jax.sharding import Mesh, NamedSharding, PartitionSpec as P
    from jax.experimental.shard_map import shard_map
    from concourse import bass2jax as b2j
    from concourse import mybir as _mb

    b2j.install_neuronx_cc_hook()
    partition_name = nc.partition_id_tensor.name if nc.partition_id_tensor else None
    in_names, out_names, out_avals = [], [], []
    for alloc in nc.m.functions[0].allocations:
        if not isinstance(alloc, _mb.MemoryLocationSet):
            continue
        name = alloc.memorylocations[0].name
        if alloc.kind == "ExternalInput":
            if name != partition_name:
                in_names.append(name)
        elif alloc.kind == "ExternalOutput":
            out_names.append(name)
            shape = tuple(alloc.tensor_shape)
            dtype = _mb.dt.np(alloc.dtype)
            out_avals.append(jax.core.ShapedArray(shape, dtype))
    all_in_names = list(in_names) + list(out_names)
    if partition_name is not None:
        all_in_names.append(partition_name)

    SHARD = P("b", None, "h", None)
    spec_by_name = {
        "x1": SHARD, "x2": SHARD, "hal": P(("b", "h"), None, None, None),
        "y": SHARD,
    }
    in_specs = tuple(spec_by_name.get(nm, P()) for nm in in_names + out_names)

    devices = jax.devices()[:8]
    mesh = Mesh(np.asarray(devices).reshape(B, 2), ("b", "h"))

    def _pjrt_body(*args):
        operands = list(args)
        if partition_name is not None:
            operands.append(b2j.partition_id_tensor())
        return tuple(b2j._bass_exec_p.bind(
            *operands, out_avals=tuple(out_avals), in_names=tuple(all_in_names),
            out_names=tuple(out_names), lowering_input_output_aliases=(),
            sim_require_finite=True, sim_require_nnan=True, nc=nc))

    sharded = jax.jit(
        shard_map(_pjrt_body, mesh=mesh, in_specs=in_specs,
                  out_specs=(SHARD,), check_rep=False),
        keep_unused=True)

    # persistent dummy for the y operand (content ignored: the NEFF's y
    # tensor is bound to the result buffer, the operand only fixes order)
    ydummy = jax.device_put(
        np.zeros((B, C, H, W), np.float32), NamedSharding(mesh, SHARD))

    def run(in_map):
        args = [in_map[nm] for nm in in_names] + [ydummy]
        return sharded(*args)[0]

    _CACHE["runner"] = run
    return run
